# revision 7
# baseline (speedup 1.0000x reference)
"""Trainium2 Bass kernel for nn_DFlashAttentionSlide (GQA attention block).

Sharding: tensor-parallel over heads across 8 NeuronCores. Core c owns
kv head c and q heads [4c, 4c+4). Activations (x/x_ctx) are replicated;
weights / kv-cache are sharded along the head dim; the output projection
is contraction-sharded, so each core returns a partial [L, HID] output
that the host sums.

Device-side layout strategy (per core):
  - projections:  q as [l, hd] (N=512 matmuls), k/v as [d, t] (N=512)
  - attention scores computed TRANSPOSED: scoresT[s, (h l)] = K @ Q^T
    with k tiles as the stationary operand and all 4 heads' q packed in
    the 512-wide moving operand.  Softmax runs on [s_tile, 512] tiles
    (exp on ACT, mask add + running sum on DVE), and the PV matmul
    consumes the exp tiles directly (contraction over s = partition dim)
    producing outT [d, (h l)] -- no probability transposes anywhere.
  - RMSNorm mean-subtract is folded into the projection weights on the
    host (mean over d of W rows per head); variance uses sum-of-squares
    via ones-matmul partition reductions; rstd broadcast back across
    partitions with a K=1 ones-matmul.
  - RoPE rotate-half is a cross-partition move done with two SBUF->SBUF
    DMA copies; the sign flip is folded into host-built sin tables.
    SCALE (1/sqrt(D)) is folded into the q-side cos/sin tables.
"""

import os
import sys

sys.path.insert(0, "/opt/trn_rl_repo")

import numpy as np
import ml_dtypes

import concourse.bass as bass
import concourse.bacc as bacc
import concourse.tile as tile
from concourse import mybir
from concourse.bass_utils import run_bass_kernel_spmd

BF16 = ml_dtypes.bfloat16

H, HKV, D, HALF = 32, 8, 128, 64
L, T, S, HID = 128, 1024, 4096, 4096
REP = H // HKV          # q heads per kv head (= per core)
EPS = 1e-6
SCALE = D ** -0.5
NCORES = 8
KT = HID // 128         # 32 contraction tiles for projections
ST = S // 128           # 32 s tiles for attention
SOLD = S - T            # 3072 cached stream positions kept
TNEW = T                # 1024 newly projected stream positions

FP32 = mybir.dt.float32
BF16_DT = mybir.dt.bfloat16

_PROGRAM_CACHE = {}

# Filled by run() when BASS_KERNEL_TRACE=1; read by test.py.
LAST_RESULTS = None


def _build_program():
    nc = bacc.Bacc("TRN2", target_bir_lowering=False, debug=False,
                   num_devices=NCORES)

    # ---- external I/O (per-core values supplied via in_maps) ----
    cT = nc.declare_dram_parameter("cT", [HID, T], BF16_DT, isOutput=False)
    wkvqT = nc.declare_dram_parameter("wkvqT", [HID, 768], BF16_DT, isOutput=False)
    woT = nc.declare_dram_parameter("woT", [REP * D, HID], BF16_DT, isOutput=False)
    ktold = nc.declare_dram_parameter("ktold", [D, SOLD], BF16_DT, isOutput=False)
    vold = nc.declare_dram_parameter("vold", [SOLD, D], BF16_DT, isOutput=False)
    maskT = nc.declare_dram_parameter("maskT", [S, L], FP32, isOutput=False)
    cosq = nc.declare_dram_parameter("cosq", [D, L], FP32, isOutput=False)
    sinq = nc.declare_dram_parameter("sinq", [D, L], FP32, isOutput=False)
    cosk = nc.declare_dram_parameter("cosk", [D, TNEW], FP32, isOutput=False)
    sink = nc.declare_dram_parameter("sink", [D, TNEW], FP32, isOutput=False)
    qw = nc.declare_dram_parameter("qw", [D, 1], FP32, isOutput=False)
    kw = nc.declare_dram_parameter("kw", [D, 1], FP32, isOutput=False)
    y = nc.declare_dram_parameter("y", [L, HID], FP32, isOutput=True)

    with tile.TileContext(nc) as tc:
        _emit(nc, tc, cT=cT, wkvqT=wkvqT, woT=woT, ktold=ktold, vold=vold,
              maskT=maskT, cosq=cosq, sinq=sinq, cosk=cosk, sink=sink,
              qw=qw, kw=kw, y=y)
    nc.compile()
    return nc


def _emit(nc, tc, *, cT, wkvqT, woT, ktold, vold, maskT, cosq, sinq,
          cosk, sink, qw, kw, y):
    from contextlib import ExitStack

    ctx = ExitStack()
    with ctx:
        # ---------------- pools ----------------
        consts = ctx.enter_context(tc.tile_pool(name="consts", bufs=1))
        streams = ctx.enter_context(tc.tile_pool(name="streams", bufs=1))
        proj_in = ctx.enter_context(tc.tile_pool(name="proj_in", bufs=3))
        normtmp = ctx.enter_context(tc.tile_pool(name="normtmp", bufs=1))
        sloop = ctx.enter_context(tc.tile_pool(name="sloop", bufs=4))
        psA = ctx.enter_context(tc.tile_pool(name="psA", bufs=1, space="PSUM"))
        psS = ctx.enter_context(tc.tile_pool(name="psS", bufs=2, space="PSUM"))
        psY = ctx.enter_context(tc.tile_pool(name="psY", bufs=1, space="PSUM"))

        # ---------------- constants ----------------
        ones_col = consts.tile([128, 1], FP32, tag="ones_col")
        nc.vector.memset(ones_col, 1.0)
        ones_row = consts.tile([1, 128], FP32, tag="ones_row")
        nc.vector.memset(ones_row, 1.0)
        eps_t = consts.tile([128, 1], FP32, tag="eps")
        nc.vector.memset(eps_t, EPS)

        qw_t = consts.tile([D, 1], FP32, tag="qw")
        nc.sync.dma_start(qw_t[:], qw[:])
        kw_t = consts.tile([D, 1], FP32, tag="kw")
        nc.sync.dma_start(kw_t[:], kw[:])
        cosq_t = consts.tile([D, L], FP32, tag="cosq")
        nc.sync.dma_start(cosq_t[:], cosq[:])
        sinq_t = consts.tile([D, L], FP32, tag="sinq")
        nc.sync.dma_start(sinq_t[:], sinq[:])
        cosk_t = consts.tile([D, TNEW], FP32, tag="cosk")
        nc.sync.dma_start(cosk_t[:], cosk[:])
        sink_t = consts.tile([D, TNEW], FP32, tag="sink")
        nc.sync.dma_start(sink_t[:], sink[:])

        # ---------------- persistent stream tensors ----------------
        # kT_stream: [d, s] bf16; v_tiles: [s-tile partitions, d] per 128-col
        # block, i.e. column block i holds v[s=i*128:(i+1)*128, :].
        kts = streams.tile([128, S], BF16_DT, tag="kts")
        nc.sync.dma_start(kts[:, 0:SOLD], ktold[:])
        vt = streams.tile([128, S], BF16_DT, tag="vt")
        nc.sync.dma_start(
            vt[:, 0:SOLD].rearrange("p (n d) -> p n d", d=128),
            vold.rearrange("(n p) d -> p n d", p=128),
        )

        # ---------------- projections ----------------
        # psum accumulators
        ps_q = psA.tile([128, 512], FP32, tag="ps_q")
        ps_k0 = psA.tile([128, 512], FP32, tag="ps_k0")
        ps_k1 = psA.tile([128, 512], FP32, tag="ps_k1")
        ps_v0 = psA.tile([128, 512], FP32, tag="ps_v0")
        ps_v1 = psA.tile([128, 512], FP32, tag="ps_v1")

        for k in range(KT):
            ct_k = proj_in.tile([128, T], BF16_DT, tag="ct")
            nc.sync.dma_start(ct_k[:], cT[k * 128:(k + 1) * 128, :])
            w_k = proj_in.tile([128, 768], BF16_DT, tag="wkvq")
            nc.sync.dma_start(w_k[:], wkvqT[k * 128:(k + 1) * 128, :])

            st = (k == 0)
            sp = (k == KT - 1)
            # k proj: [d, t] = WkT.T @ cT
            nc.tensor.matmul(ps_k0[:], w_k[:, 0:128], ct_k[:, 0:512],
                             start=st, stop=sp)
            nc.tensor.matmul(ps_k1[:], w_k[:, 0:128], ct_k[:, 512:1024],
                             start=st, stop=sp)
            # v proj: [d, t]
            nc.tensor.matmul(ps_v0[:], w_k[:, 128:256], ct_k[:, 0:512],
                             start=st, stop=sp)
            nc.tensor.matmul(ps_v1[:], w_k[:, 128:256], ct_k[:, 512:1024],
                             start=st, stop=sp)
            # q proj: [l, hd] = x.T.T @ WqT   (x.T = cT cols of the last L
            # tokens)
            nc.tensor.matmul(ps_q[:], ct_k[:, T - L:T], w_k[:, 256:768],
                             start=st, stop=sp)

        # copy psum accumulators out (frees the 5 proj banks for reuse)
        kc = normtmp.tile([128, TNEW], FP32, tag="kc")
        nc.vector.tensor_copy(kc[:, 0:512], ps_k0[:])
        nc.vector.tensor_copy(kc[:, 512:1024], ps_k1[:])
        vsb = normtmp.tile([128, TNEW], BF16_DT, tag="vsb")
        nc.vector.tensor_copy(vsb[:, 0:512], ps_v0[:])
        nc.vector.tensor_copy(vsb[:, 512:1024], ps_v1[:])
        qsb = normtmp.tile([128, 512], FP32, tag="qsb")
        nc.vector.tensor_copy(qsb[:], ps_q[:])

        # ---------------- k norm + rope -> kT_stream[:, SOLD:] ----------------
        ksq = normtmp.tile([128, TNEW], FP32, tag="ksq")
        nc.vector.tensor_mul(ksq[:], kc[:], kc[:])
        ps_sos0 = psA.tile([1, 512], FP32, tag="ps_v0")
        ps_sos1 = psA.tile([1, 512], FP32, tag="ps_v1")
        nc.tensor.matmul(ps_sos0[:], ones_col[:], ksq[:, 0:512])
        nc.tensor.matmul(ps_sos1[:], ones_col[:], ksq[:, 512:1024])
        krstd = normtmp.tile([1, TNEW], FP32, tag="krstd")
        nc.scalar.activation(krstd[:, 0:512], ps_sos0[:],
                             mybir.ActivationFunctionType.Sqrt,
                             bias=eps_t[0:1, :], scale=1.0 / D)
        nc.scalar.activation(krstd[:, 512:1024], ps_sos1[:],
                             mybir.ActivationFunctionType.Sqrt,
                             bias=eps_t[0:1, :], scale=1.0 / D)
        nc.vector.reciprocal(krstd[:], krstd[:])
        ps_krb0 = psA.tile([128, 512], FP32, tag="ps_k0")
        ps_krb1 = psA.tile([128, 512], FP32, tag="ps_k1")
        nc.tensor.matmul(ps_krb0[:], ones_row[:], krstd[:, 0:512])
        nc.tensor.matmul(ps_krb1[:], ones_row[:], krstd[:, 512:1024])
        # knw = (kc * kw) * rstd_bcast
        knw = normtmp.tile([128, TNEW], FP32, tag="knw")
        nc.vector.scalar_tensor_tensor(knw[:, 0:512], kc[:, 0:512], kw_t[:],
                                       ps_krb0[:], op0=mybir.AluOpType.mult,
                                       op1=mybir.AluOpType.mult)
        nc.vector.scalar_tensor_tensor(knw[:, 512:1024], kc[:, 512:1024],
                                       kw_t[:], ps_krb1[:],
                                       op0=mybir.AluOpType.mult,
                                       op1=mybir.AluOpType.mult)
        # rotate-half via partition-shifted SBUF copy (sign folded into sink)
        krot = normtmp.tile([128, TNEW], FP32, tag="krot")
        nc.sync.dma_start(krot[0:HALF, :], knw[HALF:D, :])
        nc.sync.dma_start(krot[HALF:D, :], knw[0:HALF, :])
        ka = normtmp.tile([128, TNEW], FP32, tag="ka")
        nc.vector.tensor_mul(ka[:], knw[:], cosk_t[:])
        kb = normtmp.tile([128, TNEW], FP32, tag="kb")
        nc.vector.tensor_mul(kb[:], krot[:], sink_t[:])
        nc.vector.tensor_add(kts[:, SOLD:S], ka[:], kb[:])

        # ---------------- v -> v_tiles[:, SOLD:] (DMA transpose) -------------
        for i in range(TNEW // 128):
            nc.sync.dma_start(vt[:, SOLD + i * 128:SOLD + (i + 1) * 128],
                              vsb[:, i * 128:(i + 1) * 128], transpose=True)

        # ---------------- q norm + rope -> qT_all [d, (h l)] -----------------
        qsq = normtmp.tile([128, 512], FP32, tag="qsq")
        nc.vector.tensor_mul(qsq[:], qsb[:], qsb[:])
        qsos = normtmp.tile([128, REP], FP32, tag="qsos")
        nc.vector.reduce_sum(
            qsos[:],
            qsq[:].rearrange("p (h l) -> p h l", h=REP),
            axis=mybir.AxisListType.X,
        )
        qrstd = normtmp.tile([128, REP], FP32, tag="qrstd")
        nc.scalar.activation(qrstd[:], qsos[:],
                             mybir.ActivationFunctionType.Sqrt,
                             bias=eps_t[:], scale=1.0 / D)
        nc.vector.reciprocal(qrstd[:], qrstd[:])
        qn = normtmp.tile([128, 512], FP32, tag="qn")
        for h in range(REP):
            nc.vector.tensor_scalar_mul(qn[:, h * 128:(h + 1) * 128],
                                        qsb[:, h * 128:(h + 1) * 128],
                                        qrstd[:, h:h + 1])
        ident = consts.tile([128, 128], FP32, tag="ident")
        from concourse.masks import make_identity
        make_identity(nc, ident[:])
        qT_all = streams.tile([128, 512], BF16_DT, tag="qT_all")
        qtw = normtmp.tile([128, 512], FP32, tag="qtw")
        for h in range(REP):
            ps_qT = psA.tile([128, 128], FP32, tag="ps_q")
            nc.tensor.transpose(ps_qT[:], qn[:, h * 128:(h + 1) * 128],
                                ident[:])
            nc.vector.tensor_scalar_mul(qtw[:, h * 128:(h + 1) * 128],
                                        ps_qT[:], qw_t[:])
        qrot = normtmp.tile([128, 512], FP32, tag="qrot")
        nc.sync.dma_start(qrot[0:HALF, :], qtw[HALF:D, :])
        nc.sync.dma_start(qrot[HALF:D, :], qtw[0:HALF, :])
        qa = normtmp.tile([128, 512], FP32, tag="qa")
        qb = normtmp.tile([128, 512], FP32, tag="qb")
        for h in range(REP):
            sl = slice(h * 128, (h + 1) * 128)
            nc.vector.tensor_mul(qa[:, sl], qtw[:, sl], cosq_t[:])
            nc.vector.tensor_mul(qb[:, sl], qrot[:, sl], sinq_t[:])
        nc.vector.tensor_add(qT_all[:], qa[:], qb[:])

        # ---------------- attention s-loop ----------------
        ps_o = psA.tile([128, 512], FP32, tag="ps_v0")
        accsum = streams.tile([128, 512], FP32, tag="accsum")
        for s in range(ST):
            ps_sc = psS.tile([128, 512], FP32, tag="ps_sc")
            nc.tensor.matmul(ps_sc[:], kts[:, s * 128:(s + 1) * 128],
                             qT_all[:])
            mk = sloop.tile([128, L], FP32, tag="mk")
            nc.sync.dma_start(mk[:], maskT[s * 128:(s + 1) * 128, :])
            sc = sloop.tile([128, 512], FP32, tag="sc")
            for h in range(REP):
                nc.vector.tensor_add(sc[:, h * 128:(h + 1) * 128],
                                     ps_sc[:, h * 128:(h + 1) * 128], mk[:])
            ex = sloop.tile([128, 512], BF16_DT, tag="ex")
            nc.scalar.activation(ex[:], sc[:],
                                 mybir.ActivationFunctionType.Exp)
            if s == 0:
                nc.vector.tensor_copy(accsum[:], ex[:])
            else:
                nc.vector.tensor_add(accsum[:], accsum[:], ex[:])
            nc.tensor.matmul(ps_o[:], vt[:, s * 128:(s + 1) * 128], ex[:],
                             start=(s == 0), stop=(s == ST - 1))

        # ---------------- normalize ----------------
        ps_sum = psA.tile([1, 512], FP32, tag="ps_v1")
        nc.tensor.matmul(ps_sum[:], ones_col[:], accsum[:])
        rec = normtmp.tile([1, 512], FP32, tag="rec")
        nc.vector.reciprocal(rec[:], ps_sum[:])
        ps_rb = psA.tile([128, 512], FP32, tag="ps_k0")
        nc.tensor.matmul(ps_rb[:], ones_row[:], rec[:])
        osb = normtmp.tile([128, 512], FP32, tag="osb")
        nc.vector.tensor_copy(osb[:], ps_o[:])
        attT = streams.tile([128, 512], BF16_DT, tag="attT")
        nc.vector.tensor_mul(attT[:], osb[:], ps_rb[:])

        # ---------------- output projection (partial) ----------------
        for e in range(HID // 512):
            ps_y = psY.tile([128, 512], FP32, tag="ps_y")
            for h in range(REP):
                wo_t = sloop.tile([128, 512], BF16_DT, tag="wo")
                nc.sync.dma_start(
                    wo_t[:],
                    woT[h * 128:(h + 1) * 128, e * 512:(e + 1) * 512])
                nc.tensor.matmul(ps_y[:], attT[:, h * 128:(h + 1) * 128],
                                 wo_t[:], start=(h == 0), stop=(h == REP - 1))
            ysb = sloop.tile([128, 512], FP32, tag="ysb")
            nc.vector.tensor_copy(ysb[:], ps_y[:])
            nc.sync.dma_start(y[:, e * 512:(e + 1) * 512], ysb[:])


def _prepare_inputs(x, x_ctx, cos_q, sin_q, cos_k, sin_k, kv_cache,
                    causal_mask, Wq, Wk, Wv, Wo, q_norm_w, k_norm_w):
    """Host-side sharding/preprocessing. Returns list of per-core in_maps."""
    f32 = np.float32
    x = np.asarray(x, f32)
    x_ctx = np.asarray(x_ctx, f32)
    c = np.concatenate([x_ctx[0], x[0]], axis=0)          # [T, HID]
    cT = np.ascontiguousarray(c.T).astype(BF16)           # [HID, T]

    maskT = np.ascontiguousarray(np.asarray(causal_mask, f32)[0, 0].T)  # [S, L]

    cosqT = np.ascontiguousarray(np.asarray(cos_q, f32)[0, 0].T) * SCALE
    sinqT_raw = np.ascontiguousarray(np.asarray(sin_q, f32)[0, 0].T)
    sinqT = sinqT_raw.copy()
    sinqT[:HALF] = -sinqT[:HALF]
    sinqT *= SCALE
    coskT = np.ascontiguousarray(np.asarray(cos_k, f32)[0, 0].T)
    sinkT = np.ascontiguousarray(np.asarray(sin_k, f32)[0, 0].T).copy()
    sinkT[:HALF] = -sinkT[:HALF]

    qwc = np.ascontiguousarray(np.asarray(q_norm_w, f32).reshape(D, 1))
    kwc = np.ascontiguousarray(np.asarray(k_norm_w, f32).reshape(D, 1))

    Wq = np.asarray(Wq, f32)
    Wk = np.asarray(Wk, f32)
    Wv = np.asarray(Wv, f32)
    Wo = np.asarray(Wo, f32)
    kv = np.asarray(kv_cache, f32)

    in_maps = []
    for cidx in range(NCORES):
        hd = slice(cidx * REP * D, (cidx + 1) * REP * D)
        wq_c = Wq[hd].reshape(REP, D, HID)
        wq_c = wq_c - wq_c.mean(axis=1, keepdims=True)    # fold mean-subtract
        wq_c = wq_c.reshape(REP * D, HID)
        wk_c = Wk[cidx * D:(cidx + 1) * D]
        wk_c = wk_c - wk_c.mean(axis=0, keepdims=True)
        wv_c = Wv[cidx * D:(cidx + 1) * D]
        wkvqT = np.concatenate([wk_c.T, wv_c.T, wq_c.T], axis=1)  # [HID, 768]
        woT = np.ascontiguousarray(Wo[:, hd].T)           # [512, HID]
        ktold = np.ascontiguousarray(kv[0, cidx, T:, :].T)  # [D, SOLD]
        vold = np.ascontiguousarray(kv[1, cidx, T:, :])     # [SOLD, D]
        in_maps.append(dict(
            cT=cT,
            wkvqT=np.ascontiguousarray(wkvqT).astype(BF16),
            woT=woT.astype(BF16),
            ktold=ktold.astype(BF16),
            vold=vold.astype(BF16),
            maskT=maskT,
            cosq=cosqT.astype(f32), sinq=sinqT.astype(f32),
            cosk=coskT.astype(f32), sink=sinkT.astype(f32),
            qw=qwc, kw=kwc,
        ))
    return in_maps


def kernel(**inputs) -> np.ndarray:
    global LAST_RESULTS
    if "nc" not in _PROGRAM_CACHE:
        _PROGRAM_CACHE["nc"] = _build_program()
    nc = _PROGRAM_CACHE["nc"]
    in_maps = _prepare_inputs(**inputs)
    trace = bool(int(os.environ.get("BASS_KERNEL_TRACE", "0")))
    res = run_bass_kernel_spmd(nc, in_maps, list(range(NCORES)), trace=trace)
    LAST_RESULTS = res
    y = np.zeros((L, HID), np.float64)
    for cidx in range(NCORES):
        y += res.results[cidx]["y"].astype(np.float64)
    return y.astype(np.float32).reshape(1, L, HID)
